# revision 6
# baseline (speedup 1.0000x reference)
"""Trainium2 Bass kernel for nn_GaussianLayer (segment_reduce).

Computes ll[b, r, k] = -0.5 * sum_d((x[b, regions[r,d]] - means[r,k,d]) / scales[r,k,d])^2
                       - sum_d log(scales[r,k,d]) - 0.5 * D * log(2*pi)

Strategy (data-parallel over batch across 8 cores, 512 rows each):
  Host folds the small [R,K,D] params into matmul weights:
      ll = Xsq @ Wsq + Xraw @ Wraw + const
  where Xraw[b, (r,d)] = x[b, regions[r,d]] (the gather), Xsq = Xraw^2,
  Wsq = -0.5/scales^2, Wraw = means/scales^2 (block-diagonal per region),
  const[r,k] = -0.5*sum_d(means^2/scales^2) - sum_d log(scales) - 0.5*D*log(2pi).

  Device per 128-row batch tile:
      DMA x -> SBUF
      gpsimd.ap_gather     : free-dim gather x[:, perm]  (perm = regions.flatten())
      cast f32->bf16 (ACT) : matmul operand dtype
      PE transpose (x8)    : [128b, 128rd] -> [128rd, 128b] blocks
      ACT square           : squared operand
      PE matmul (x16)      : block-diag weights, 2 region-groups (256 out cols) per matmul
      DVE add const        : PSUM + const_tile -> SBUF (broadcast const over batch rows)
      DMA out
"""

import os
import sys

for _p in ("/opt/trn_rl_repo", "/root/.axon_site/_ro/trn_rl_repo"):
    if os.path.isdir(_p) and _p not in sys.path:
        sys.path.insert(0, _p)

import numpy as np
import ml_dtypes

import concourse.bass as bass
import concourse.tile as tile
from concourse import bacc, library_config, mybir
from concourse.bass_utils import run_bass_kernel_spmd

LOG_2PI = 1.8378770664093453
B, F = 4096, 1024
R, K, D = 64, 32, 16
NCORES = 8
BL = B // NCORES      # 512 batch rows per core
NT = BL // 128        # 4 batch tiles per core
RKCOLS = R * K        # 2048 output columns
NPAIR = 8             # pair = 2 region-groups = 8 regions = 128 gathered rows / 256 out cols

_module_cache = {}


def _build_module():
    if "nc" in _module_cache:
        return _module_cache["nc"]

    nc = bacc.Bacc(
        trn_type="TRN2",
        target_bir_lowering=False,
        debug=False,
        enable_asserts=False,
    )
    bf16 = mybir.dt.bfloat16
    f32 = mybir.dt.float32
    i16 = mybir.dt.int16

    x_d = nc.dram_tensor("x", [BL, F], f32, kind="ExternalInput").ap()
    wraw_d = nc.dram_tensor("wraw", [128, RKCOLS], bf16, kind="ExternalInput").ap()
    wsq_d = nc.dram_tensor("wsq", [128, RKCOLS], bf16, kind="ExternalInput").ap()
    const_d = nc.dram_tensor("cst", [128, RKCOLS], f32, kind="ExternalInput").ap()
    idx_d = nc.dram_tensor("idx", [128, F // 16], i16, kind="ExternalInput").ap()
    id_d = nc.dram_tensor("ident", [128, 128], bf16, kind="ExternalInput").ap()
    out_d = nc.dram_tensor("out", [BL, RKCOLS], f32, kind="ExternalOutput").ap()

    with tile.TileContext(nc) as tc:
        with (
            tc.tile_pool(name="persist", bufs=1) as persist,
            tc.tile_pool(name="xin", bufs=3) as xpool,
            tc.tile_pool(name="xg", bufs=2) as xgpool,
            tc.tile_pool(name="xgb", bufs=2) as xgbpool,
            tc.tile_pool(name="trp", bufs=2, space="PSUM") as trpool,
            tc.tile_pool(name="raw", bufs=3) as rawpool,
            tc.tile_pool(name="sq", bufs=3) as sqpool,
            tc.tile_pool(name="po", bufs=3, space="PSUM") as popool,
            tc.tile_pool(name="osb", bufs=2) as opool,
        ):
            nc.gpsimd.load_library(library_config.ap_gather)

            w_raw = persist.tile([128, RKCOLS], bf16)
            nc.sync.dma_start(w_raw[:], wraw_d)
            w_sq = persist.tile([128, RKCOLS], bf16)
            nc.sync.dma_start(w_sq[:], wsq_d)
            cst = persist.tile([128, RKCOLS], f32)
            nc.sync.dma_start(cst[:], const_d)
            idx = persist.tile([128, F // 16], i16)
            nc.sync.dma_start(idx[:], idx_d)
            ident = persist.tile([128, 128], bf16)
            nc.sync.dma_start(ident[:], id_d)

            for bt in range(NT):
                rs = slice(bt * 128, (bt + 1) * 128)
                xt = xpool.tile([128, F], f32)
                nc.sync.dma_start(xt[:], x_d[rs, :])

                xg = xgpool.tile([128, F], f32)
                nc.gpsimd.ap_gather(
                    xg[:], xt[:], idx[:],
                    channels=128, num_elems=F, d=1, num_idxs=F,
                )

                xgb = xgbpool.tile([128, F], bf16)
                nc.scalar.copy(xgb[:], xg[:])

                raws, sqs = [], []
                for c4 in range(2):
                    pt = trpool.tile([128, 512], bf16)
                    for jj in range(4):
                        j = 4 * c4 + jj
                        nc.tensor.transpose(
                            pt[:, jj * 128:(jj + 1) * 128],
                            xgb[:, j * 128:(j + 1) * 128],
                            ident[:],
                        )
                    raw = rawpool.tile([128, 512], bf16)
                    nc.vector.tensor_copy(raw[:], pt[:])
                    sq = sqpool.tile([128, 512], bf16)
                    nc.scalar.square(sq[:], raw[:])
                    raws.append(raw)
                    sqs.append(sq)

                osb = opool.tile([128, RKCOLS], f32)
                for q in range(4):
                    po = popool.tile([128, 512], f32)
                    for h in range(2):
                        p = 2 * q + h
                        raw, sq = raws[p // 4], sqs[p // 4]
                        sl = slice((p % 4) * 128, (p % 4 + 1) * 128)
                        co = slice(h * 256, (h + 1) * 256)
                        wc = slice(p * 256, (p + 1) * 256)
                        nc.tensor.matmul(
                            po[:, co], raw[:, sl], w_raw[:, wc],
                            start=True, stop=False,
                        )
                        nc.tensor.matmul(
                            po[:, co], sq[:, sl], w_sq[:, wc],
                            start=False, stop=True,
                        )
                    cs = slice(q * 512, (q + 1) * 512)
                    nc.vector.tensor_add(osb[:, cs], po[:], cst[:, cs])

                nc.sync.dma_start(out_d[rs, :], osb[:])

    nc.compile()
    _module_cache["nc"] = nc
    return nc


def _prep_params(regions, means, scales):
    """Host folding of the small [R,K,D] params into matmul weights."""
    regions = np.asarray(regions).astype(np.int64)
    means = np.asarray(means, dtype=np.float64)
    scales = np.asarray(scales, dtype=np.float64)

    inv2 = 1.0 / scales**2                                   # [R,K,D]
    wsq_c = -0.5 * inv2                                      # coeff of x^2
    wraw_c = means * inv2                                    # coeff of x  (= -0.5 * (-2 m / s^2))
    const = (
        -0.5 * np.sum(means**2 * inv2, axis=-1)
        - np.sum(np.log(scales), axis=-1)
        - 0.5 * D * LOG_2PI
    )                                                        # [R,K]

    # Block-diagonal weight tiles: pair p covers regions 8p..8p+7.
    # Row 16j+d (region-local j in 0..7), col 32j+k.
    wraw = np.zeros((128, RKCOLS), np.float32)
    wsq = np.zeros((128, RKCOLS), np.float32)
    for p in range(NPAIR):
        for j in range(8):
            r = 8 * p + j
            rows = slice(16 * j, 16 * j + 16)
            cols = slice(256 * p + 32 * j, 256 * p + 32 * j + 32)
            wraw[rows, cols] = wraw_c[r].T.astype(np.float32)   # [D, K]
            wsq[rows, cols] = wsq_c[r].T.astype(np.float32)
    wraw = wraw.astype(ml_dtypes.bfloat16)
    wsq = wsq.astype(ml_dtypes.bfloat16)

    const_tile = np.broadcast_to(
        const.reshape(-1).astype(np.float32), (128, RKCOLS)
    ).copy()

    # ap_gather index layout: index j lives at [j % 16, j // 16], replicated
    # across the eight 16-partition groups.
    perm = regions.reshape(-1).astype(np.int16)              # [1024]
    idx16 = perm.reshape(F // 16, 16).T                      # [16, 64]
    idx = np.tile(idx16, (8, 1)).copy()                      # [128, 64]

    ident = np.eye(128, dtype=ml_dtypes.bfloat16)
    return wraw, wsq, const_tile, idx, ident


def _run(inputs, trace=False, **kwargs):
    x = np.ascontiguousarray(np.asarray(inputs["x"], dtype=np.float32))
    assert x.shape == (B, F), x.shape
    wraw, wsq, const_tile, idx, ident = _prep_params(
        inputs["regions"], inputs["means"], inputs["scales"]
    )

    nc = _build_module()
    in_maps = []
    for c in range(NCORES):
        in_maps.append({
            "x": np.ascontiguousarray(x[c * BL:(c + 1) * BL]),
            "wraw": wraw,
            "wsq": wsq,
            "cst": const_tile,
            "idx": idx,
            "ident": ident,
        })
    res = run_bass_kernel_spmd(
        nc, in_maps, core_ids=list(range(NCORES)), trace=trace, **kwargs
    )
    out = np.concatenate(
        [res.results[c]["out"] for c in range(NCORES)], axis=0
    ).reshape(B, R, K)
    return out, res


def kernel(**inputs):
    out, _ = _run(inputs, trace=False)
    return out


# revision 7
# speedup vs baseline: 2.2938x; 2.2938x over previous
"""Trainium2 Bass kernel for nn_GaussianLayer (segment_reduce).

Computes ll[b, r, k] = -0.5 * sum_d((x[b, regions[r,d]] - means[r,k,d]) / scales[r,k,d])^2
                       - sum_d log(scales[r,k,d]) - 0.5 * D * log(2*pi)

Strategy (data-parallel over batch across 8 cores, 512 rows each):
  Host folds the small [R,K,D] params into matmul weights:
      ll = Xsq @ Wsq + Xraw @ Wraw + const
  where Xraw[b, (r,d)] = x[b, regions[r,d]] (the gather), Xsq = Xraw^2,
  Wsq = -0.5/scales^2, Wraw = means/scales^2 (block-diagonal per region),
  const[r,k] = -0.5*sum_d(means^2/scales^2) - sum_d log(scales) - 0.5*D*log(2pi).

  Device, per core:
    phase 1 (per 128-row batch tile): DMA x -> cast bf16 (ACT) ->
        PE-transpose 8x [128,128] -> xT[1024 features, 512 batch] bf16 -> HBM scratch
    phase 2: 8x gpsimd.dma_gather pulls 128 gathered feature-rows each
        (region order) straight into SBUF as the matmul lhsT tiles
    phase 3: ACT square, PE matmuls vs block-diagonal weights
        (2 region-groups / 256 out cols per matmul), DVE const-add, DMA out.
"""

import os
import sys

for _p in ("/opt/trn_rl_repo", "/root/.axon_site/_ro/trn_rl_repo"):
    if os.path.isdir(_p) and _p not in sys.path:
        sys.path.insert(0, _p)

import numpy as np
import ml_dtypes

import concourse.bass as bass
import concourse.tile as tile
from concourse import bacc, library_config, mybir
from concourse.bass_utils import run_bass_kernel_spmd

LOG_2PI = 1.8378770664093453
B, F = 4096, 1024
R, K, D = 64, 32, 16
NCORES = 8
BL = B // NCORES      # 512 batch rows per core
NT = BL // 128        # 4 batch tiles per core
RKCOLS = R * K        # 2048 output columns
NPAIR = 8             # pair = 2 region-groups = 8 regions = 128 gathered rows / 256 out cols
N_WARM = 24           # dummy matmuls to lift the PE HAM clock-gate early

_module_cache = {}


def _build_module():
    if "nc" in _module_cache:
        return _module_cache["nc"]

    nc = bacc.Bacc(
        trn_type="TRN2",
        target_bir_lowering=False,
        debug=False,
        enable_asserts=False,
    )
    bf16 = mybir.dt.bfloat16
    f32 = mybir.dt.float32
    i16 = mybir.dt.int16

    x_d = nc.dram_tensor("x", [BL, F], f32, kind="ExternalInput").ap()
    wraw_d = nc.dram_tensor("wraw", [128, RKCOLS], bf16, kind="ExternalInput").ap()
    wsq_d = nc.dram_tensor("wsq", [128, RKCOLS], bf16, kind="ExternalInput").ap()
    const_d = nc.dram_tensor("cst", [1, RKCOLS], f32, kind="ExternalInput").ap()
    idx_d = nc.dram_tensor("idx", [128, F // 16], i16, kind="ExternalInput").ap()
    id_d = nc.dram_tensor("ident", [128, 128], bf16, kind="ExternalInput").ap()
    out_d = nc.dram_tensor("out", [BL, RKCOLS], f32, kind="ExternalOutput").ap()

    with tile.TileContext(nc) as tc:
        with (
            tc.tile_pool(name="persist", bufs=1) as persist,
            tc.tile_pool(name="dram", bufs=1, space="DRAM") as drampool,
            tc.tile_pool(name="xin", bufs=3) as xpool,
            tc.tile_pool(name="xgb", bufs=2) as xgbpool,
            tc.tile_pool(name="trp", bufs=2, space="PSUM") as trpool,
            tc.tile_pool(name="wrm", bufs=1, space="PSUM") as warmpool,
            tc.tile_pool(name="xts", bufs=2) as xtspool,
            tc.tile_pool(name="gt", bufs=1) as gtpool,
            tc.tile_pool(name="sq", bufs=1) as sqpool,
            tc.tile_pool(name="po", bufs=3, space="PSUM") as popool,
            tc.tile_pool(name="osb", bufs=2) as opool,
        ):
            nc.gpsimd.load_library(library_config.mlp)

            w_raw = persist.tile([128, RKCOLS], bf16)
            nc.sync.dma_start(w_raw[:], wraw_d)
            w_sq = persist.tile([128, RKCOLS], bf16)
            nc.sync.dma_start(w_sq[:], wsq_d)
            cst1 = persist.tile([1, RKCOLS], f32)
            nc.sync.dma_start(cst1[:], const_d)
            cst = persist.tile([128, RKCOLS], f32)
            nc.gpsimd.partition_broadcast(cst[:], cst1[:])
            idx = persist.tile([128, F // 16], i16)
            nc.sync.dma_start(idx[:], idx_d)
            ident = persist.tile([128, 128], bf16)
            nc.sync.dma_start(ident[:], id_d)

            # HBM scratch holding xT (feature-major, bf16): row f = 512 batch vals
            xt_dram = drampool.tile([F, BL], bf16)
            # row f lives at [partition f%128, chunk f//128] during the write
            xt_wview = xt_dram[:].rearrange("(c p) b -> p c b", p=128)

            # PE warm-up: harmless matmuls to flip HAM to 8/8 while DMAs run
            warm = warmpool.tile([128, 512], f32)
            for _ in range(N_WARM):
                nc.tensor.matmul(warm[:, 0:256], ident[:], w_raw[:, 0:256],
                                 start=True, stop=True)

            # ---- phase 1: transpose x into xT (HBM) ----
            for bt in range(NT):
                rs = slice(bt * 128, (bt + 1) * 128)
                xt = xpool.tile([128, F], f32)
                nc.sync.dma_start(xt[:], x_d[rs, :])
                xgb = xgbpool.tile([128, F], bf16)
                nc.scalar.copy(xgb[:], xt[:])

                xts = xtspool.tile([128, F], bf16)  # [128, 8 chunks, 128 b]
                for half in range(2):
                    pt = trpool.tile([128, 512], bf16)
                    for jj in range(4):
                        c = 4 * half + jj
                        nc.tensor.transpose(
                            pt[:, jj * 128:(jj + 1) * 128],
                            xgb[:, c * 128:(c + 1) * 128],
                            ident[:],
                        )
                    nc.vector.tensor_copy(
                        xts[:, half * 512:(half + 1) * 512], pt[:]
                    )
                nc.sync.dma_start(
                    xt_wview[:, :, bt * 128:(bt + 1) * 128],
                    xts[:].rearrange("p (c b) -> p c b", c=8),
                )

            # ---- phase 2: gather region-ordered feature rows ----
            gts, sqs = [], []
            for p in range(NPAIR):
                gt = gtpool.tile([128, BL], bf16, tag=f"gt{p}")
                nc.gpsimd.dma_gather(
                    out_ap=gt[:].rearrange("p (a b) -> p a b", a=1),
                    in_ap=xt_dram[:].rearrange("(a f) b -> a f b", a=1)[0],
                    idxs_ap=idx[:, p * 8:(p + 1) * 8],
                    num_idxs=128,
                    num_idxs_reg=128,
                    elem_size=BL,
                )
                sq = sqpool.tile([128, BL], bf16, tag=f"sq{p}")
                nc.scalar.square(sq[:], gt[:])
                gts.append(gt)
                sqs.append(sq)

            # ---- phase 3: block-diag matmuls + const add + store ----
            for bt in range(NT):
                rs = slice(bt * 128, (bt + 1) * 128)
                bs = slice(bt * 128, (bt + 1) * 128)
                osb = opool.tile([128, RKCOLS], f32)
                for q in range(4):
                    po = popool.tile([128, 512], f32)
                    for h in range(2):
                        p = 2 * q + h
                        co = slice(h * 256, (h + 1) * 256)
                        wc = slice(p * 256, (p + 1) * 256)
                        nc.tensor.matmul(
                            po[:, co], gts[p][:, bs], w_raw[:, wc],
                            start=True, stop=False,
                        )
                        nc.tensor.matmul(
                            po[:, co], sqs[p][:, bs], w_sq[:, wc],
                            start=False, stop=True,
                        )
                    cs = slice(q * 512, (q + 1) * 512)
                    nc.vector.tensor_add(osb[:, cs], po[:], cst[:, cs])
                nc.sync.dma_start(out_d[rs, :], osb[:])

    nc.compile()
    _module_cache["nc"] = nc
    return nc


def _prep_params(regions, means, scales):
    """Host folding of the small [R,K,D] params into matmul weights."""
    regions = np.asarray(regions).astype(np.int64)
    means = np.asarray(means, dtype=np.float64)
    scales = np.asarray(scales, dtype=np.float64)

    inv2 = 1.0 / scales**2                                   # [R,K,D]
    wsq_c = -0.5 * inv2                                      # coeff of x^2
    wraw_c = means * inv2                                    # coeff of x
    const = (
        -0.5 * np.sum(means**2 * inv2, axis=-1)
        - np.sum(np.log(scales), axis=-1)
        - 0.5 * D * LOG_2PI
    )                                                        # [R,K]

    # Block-diagonal weight tiles: pair p covers regions 8p..8p+7.
    # Row 16j+d (region-local j in 0..7), col 32j+k.
    wraw = np.zeros((128, RKCOLS), np.float32)
    wsq = np.zeros((128, RKCOLS), np.float32)
    for p in range(NPAIR):
        for j in range(8):
            r = 8 * p + j
            rows = slice(16 * j, 16 * j + 16)
            cols = slice(256 * p + 32 * j, 256 * p + 32 * j + 32)
            wraw[rows, cols] = wraw_c[r].T.astype(np.float32)   # [D, K]
            wsq[rows, cols] = wsq_c[r].T.astype(np.float32)
    wraw = wraw.astype(ml_dtypes.bfloat16)
    wsq = wsq.astype(ml_dtypes.bfloat16)

    const_row = const.reshape(1, -1).astype(np.float32).copy()

    # dma_gather index layout: index j of a 128-row gather lives at
    # [j % 16, j // 16], replicated across the eight 16-partition groups.
    perm = regions.reshape(-1).astype(np.int16)              # [1024]
    idx16 = perm.reshape(F // 16, 16).T                      # [16, 64]
    idx = np.tile(idx16, (8, 1)).copy()                      # [128, 64]

    ident = np.eye(128, dtype=ml_dtypes.bfloat16)
    return wraw, wsq, const_row, idx, ident


def _run(inputs, trace=False, **kwargs):
    x = np.ascontiguousarray(np.asarray(inputs["x"], dtype=np.float32))
    assert x.shape == (B, F), x.shape
    wraw, wsq, const_row, idx, ident = _prep_params(
        inputs["regions"], inputs["means"], inputs["scales"]
    )

    nc = _build_module()
    in_maps = []
    for c in range(NCORES):
        in_maps.append({
            "x": np.ascontiguousarray(x[c * BL:(c + 1) * BL]),
            "wraw": wraw,
            "wsq": wsq,
            "cst": const_row,
            "idx": idx,
            "ident": ident,
        })
    res = run_bass_kernel_spmd(
        nc, in_maps, core_ids=list(range(NCORES)), trace=trace, **kwargs
    )
    out = np.concatenate(
        [res.results[c]["out"] for c in range(NCORES)], axis=0
    ).reshape(B, R, K)
    return out, res


def kernel(**inputs):
    out, _ = _run(inputs, trace=False)
    return out


# revision 13
# speedup vs baseline: 2.4646x; 1.0745x over previous
"""Trainium2 Bass kernel for nn_GaussianLayer (segment_reduce).

Computes ll[b, r, k] = -0.5 * sum_d((x[b, regions[r,d]] - means[r,k,d]) / scales[r,k,d])^2
                       - sum_d log(scales[r,k,d]) - 0.5 * D * log(2*pi)

Strategy (data-parallel over batch across 8 cores, 512 rows each):
  Host folds the small [R,K,D] params into matmul weights:
      ll = Xsq @ Wsq + Xraw @ Wraw + const
  where Xraw[b, (r,d)] = x[b, regions[r,d]] (the gather), Xsq = Xraw^2,
  Wsq = -0.5/scales^2, Wraw = means/scales^2 (block-diagonal per region),
  const[r,k] = -0.5*sum_d(means^2/scales^2) - sum_d log(scales) - 0.5*D*log(2pi).

  Device, per core:
    phase 1 (per 128-row batch tile): DMA x -> cast bf16 (ACT) ->
        PE-transpose 8x [128,128] -> xT[1024 features, 512 batch] bf16 -> HBM scratch
    phase 2: 8x gpsimd.dma_gather pulls 128 gathered feature-rows each
        (region order) straight into SBUF as the matmul lhsT tiles
    phase 3: ACT square, PE matmuls vs block-diagonal weights
        (2 region-groups / 256 out cols per matmul), DVE const-add, DMA out.
"""

import os
import sys

for _p in ("/opt/trn_rl_repo", "/root/.axon_site/_ro/trn_rl_repo"):
    if os.path.isdir(_p) and _p not in sys.path:
        sys.path.insert(0, _p)

import numpy as np
import ml_dtypes

import concourse.bass as bass
import concourse.tile as tile
from concourse import bacc, library_config, mybir
from concourse.bass_utils import run_bass_kernel_spmd

LOG_2PI = 1.8378770664093453
B, F = 4096, 1024
R, K, D = 64, 32, 16
NCORES = 8
BL = B // NCORES      # 512 batch rows per core
NT = BL // 128        # 4 batch tiles per core
RKCOLS = R * K        # 2048 output columns
NPAIR = 8             # pair = 2 region-groups = 8 regions = 128 gathered rows / 256 out cols
N_WARM = 24           # dummy matmuls to lift the PE HAM clock-gate early

_module_cache = {}


def _build_module():
    if "nc" in _module_cache:
        return _module_cache["nc"]

    nc = bacc.Bacc(
        trn_type="TRN2",
        target_bir_lowering=False,
        debug=False,
        enable_asserts=False,
    )
    bf16 = mybir.dt.bfloat16
    f32 = mybir.dt.float32
    i16 = mybir.dt.int16

    x_d = nc.dram_tensor("x", [BL, F], f32, kind="ExternalInput").ap()
    wraw_d = nc.dram_tensor("wraw", [128, RKCOLS], bf16, kind="ExternalInput").ap()
    wsq_d = nc.dram_tensor("wsq", [128, RKCOLS], bf16, kind="ExternalInput").ap()
    const_d = nc.dram_tensor("cst", [1, RKCOLS], f32, kind="ExternalInput").ap()
    idx_d = nc.dram_tensor("idx", [128, F // 16], i16, kind="ExternalInput").ap()
    id_d = nc.dram_tensor("ident", [128, 128], bf16, kind="ExternalInput").ap()
    out_d = nc.dram_tensor("out", [BL, RKCOLS], f32, kind="ExternalOutput").ap()

    with tile.TileContext(nc) as tc:
        with (
            tc.tile_pool(name="persist", bufs=1) as persist,
            tc.tile_pool(name="dram", bufs=1, space="DRAM") as drampool,
            tc.tile_pool(name="xin", bufs=3) as xpool,
            tc.tile_pool(name="xgb", bufs=2) as xgbpool,
            tc.tile_pool(name="trp", bufs=2, space="PSUM") as trpool,
            tc.tile_pool(name="wrm", bufs=1, space="PSUM") as warmpool,
            tc.tile_pool(name="xts", bufs=2) as xtspool,
            tc.tile_pool(name="gt", bufs=1) as gtpool,
            tc.tile_pool(name="sq", bufs=1) as sqpool,
            tc.tile_pool(name="po", bufs=3, space="PSUM") as popool,
            tc.tile_pool(name="osb", bufs=2) as opool,
        ):
            nc.gpsimd.load_library(library_config.mlp)

            w_raw = persist.tile([128, RKCOLS], bf16)
            nc.sync.dma_start(w_raw[:], wraw_d)
            w_sq = persist.tile([128, RKCOLS], bf16)
            nc.sync.dma_start(w_sq[:], wsq_d)
            cst1 = persist.tile([1, RKCOLS], f32)
            nc.sync.dma_start(cst1[:], const_d)
            cst = persist.tile([128, RKCOLS], f32)
            idx = persist.tile([128, F // 16], i16)
            nc.sync.dma_start(idx[:], idx_d)
            ident = persist.tile([128, 128], bf16)
            nc.sync.dma_start(ident[:], id_d)

            # HBM scratch holding xT (feature-major, bf16): row f = 512 batch vals
            xt_dram = drampool.tile([F, BL], bf16)
            # row f lives at [partition f%128, chunk f//128] during the write
            xt_wview = xt_dram[:].rearrange("(c p) b -> p c b", p=128)

            # ---- phase 1: transpose x into xT (HBM) ----
            warm = warmpool.tile([128, 512], f32)
            for bt in range(NT):
                rs = slice(bt * 128, (bt + 1) * 128)
                xt = xpool.tile([128, F], f32)
                nc.sync.dma_start(xt[:], x_d[rs, :])
                xgb = xgbpool.tile([128, F], bf16)
                nc.scalar.copy(xgb[:], xt[:])

                xts = xtspool.tile([128, F], bf16)  # [128, 8 chunks, 128 b]
                for half in range(2):
                    pt = trpool.tile([128, 512], bf16)
                    for jj in range(4):
                        c = 4 * half + jj
                        nc.tensor.transpose(
                            pt[:, jj * 128:(jj + 1) * 128],
                            xgb[:, c * 128:(c + 1) * 128],
                            ident[:],
                        )
                    nc.vector.tensor_copy(
                        xts[:, half * 512:(half + 1) * 512], pt[:]
                    )
                nc.sync.dma_start(
                    xt_wview[:, :, bt * 128:(bt + 1) * 128],
                    xts[:].rearrange("p (c b) -> p c b", c=8),
                )
                # PE warm-up reading this tile: keeps HAM at 8/8 through the
                # gather window so phase-3 matmuls run at 2.4 GHz
                for _ in range(N_WARM // NT):
                    nc.tensor.matmul(warm[:, 0:256], xts[:, 0:128],
                                     w_raw[:, 0:256], start=True, stop=True)

            # ---- phase 2: gather region-ordered feature rows ----
            gts, sqs = [], []
            for p in range(NPAIR):
                gt = gtpool.tile([128, BL], bf16, tag=f"gt{p}")
                nc.gpsimd.dma_gather(
                    out_ap=gt[:].rearrange("p (a b) -> p a b", a=1),
                    in_ap=xt_dram[:].rearrange("(a f) b -> a f b", a=1)[0],
                    idxs_ap=idx[:, p * 8:(p + 1) * 8],
                    num_idxs=128,
                    num_idxs_reg=128,
                    elem_size=BL,
                )
                sq = sqpool.tile([128, BL], bf16, tag=f"sq{p}")
                nc.vector.tensor_mul(sq[:], gt[:], gt[:])
                gts.append(gt)
                sqs.append(sq)
            # const broadcast sits on gpsimd too: emit it after the gathers so
            # it does not delay them (consumed only by late phase-3 adds)
            nc.gpsimd.partition_broadcast(cst[:], cst1[:])

            # ---- phase 3: block-diag matmuls + const add + store ----
            for bt in range(NT):
                rs = slice(bt * 128, (bt + 1) * 128)
                bs = slice(bt * 128, (bt + 1) * 128)
                osb = opool.tile([128, RKCOLS], f32)
                for q in range(4):
                    po = popool.tile([128, 512], f32)
                    for h in range(2):
                        p = 2 * q + h
                        co = slice(h * 256, (h + 1) * 256)
                        wc = slice(p * 256, (p + 1) * 256)
                        nc.tensor.matmul(
                            po[:, co], gts[p][:, bs], w_raw[:, wc],
                            start=True, stop=False,
                        )
                        nc.tensor.matmul(
                            po[:, co], sqs[p][:, bs], w_sq[:, wc],
                            start=False, stop=True,
                        )
                    cs = slice(q * 512, (q + 1) * 512)
                    nc.vector.tensor_add(osb[:, cs], po[:], cst[:, cs])
                    if q == 1:
                        nc.sync.dma_start(out_d[rs, 0:1024], osb[:, 0:1024])
                nc.sync.dma_start(out_d[rs, 1024:2048], osb[:, 1024:2048])

    nc.compile()
    _module_cache["nc"] = nc
    return nc


def _prep_params(regions, means, scales):
    """Host folding of the small [R,K,D] params into matmul weights."""
    regions = np.asarray(regions).astype(np.int64)
    means = np.asarray(means, dtype=np.float64)
    scales = np.asarray(scales, dtype=np.float64)

    inv2 = 1.0 / scales**2                                   # [R,K,D]
    wsq_c = -0.5 * inv2                                      # coeff of x^2
    wraw_c = means * inv2                                    # coeff of x
    const = (
        -0.5 * np.sum(means**2 * inv2, axis=-1)
        - np.sum(np.log(scales), axis=-1)
        - 0.5 * D * LOG_2PI
    )                                                        # [R,K]

    # Block-diagonal weight tiles: pair p covers regions 8p..8p+7.
    # Row 16j+d (region-local j in 0..7), col 32j+k.
    wraw = np.zeros((128, RKCOLS), np.float32)
    wsq = np.zeros((128, RKCOLS), np.float32)
    for p in range(NPAIR):
        for j in range(8):
            r = 8 * p + j
            rows = slice(16 * j, 16 * j + 16)
            cols = slice(256 * p + 32 * j, 256 * p + 32 * j + 32)
            wraw[rows, cols] = wraw_c[r].T.astype(np.float32)   # [D, K]
            wsq[rows, cols] = wsq_c[r].T.astype(np.float32)
    wraw = wraw.astype(ml_dtypes.bfloat16)
    wsq = wsq.astype(ml_dtypes.bfloat16)

    const_row = const.reshape(1, -1).astype(np.float32).copy()

    # dma_gather index layout: index j of a 128-row gather lives at
    # [j % 16, j // 16], replicated across the eight 16-partition groups.
    perm = regions.reshape(-1).astype(np.int16)              # [1024]
    idx16 = perm.reshape(F // 16, 16).T                      # [16, 64]
    idx = np.tile(idx16, (8, 1)).copy()                      # [128, 64]

    ident = np.eye(128, dtype=ml_dtypes.bfloat16)
    return wraw, wsq, const_row, idx, ident


def _run(inputs, trace=False, **kwargs):
    x = np.ascontiguousarray(np.asarray(inputs["x"], dtype=np.float32))
    assert x.shape == (B, F), x.shape
    wraw, wsq, const_row, idx, ident = _prep_params(
        inputs["regions"], inputs["means"], inputs["scales"]
    )

    nc = _build_module()
    in_maps = []
    for c in range(NCORES):
        in_maps.append({
            "x": np.ascontiguousarray(x[c * BL:(c + 1) * BL]),
            "wraw": wraw,
            "wsq": wsq,
            "cst": const_row,
            "idx": idx,
            "ident": ident,
        })
    res = run_bass_kernel_spmd(
        nc, in_maps, core_ids=list(range(NCORES)), trace=trace, **kwargs
    )
    out = np.concatenate(
        [res.results[c]["out"] for c in range(NCORES)], axis=0
    ).reshape(B, R, K)
    return out, res


def kernel(**inputs):
    out, _ = _run(inputs, trace=False)
    return out
